# revision 20
# baseline (speedup 1.0000x reference)
"""CrossViewAttention Trainium2 kernel (v2).

Shards the B*V=16 (batch, view) attention instances across 8 NeuronCores,
2 per core, paired as (b, v) and (b, v+2) so the two instances share KV
source view v+1 (each instance attends over views v-1, v+1 circular).
Per core the 3 distinct KV source views are projected once (25% fewer
K/V projection FLOPs vs the naive 4). All matmul operands are bf16
(fp32 PSUM accumulation); rel-err budget is 2e-2 and bf16 lands ~3e-3.

Pipeline (single rotating 8-bank PSUM pool, no phase barriers):
  A1  K^T = wk^T @ x_kv^T     [feat, t]  (ACT drains psum -> KT bf16)
  A2  V   = x_kv @ wv          [t, feat]  (+ ones col -> VA, DVE drain)
  A3+B interleaved per head-pair j (software pipeline, lag 1):
      emit QK(j-1) -> emit A3(j) -> emit PV(j-1)
      so the ACT-engine exp of step j-1 hides under A3(j) matmuls.
      PV's ones-row yields softmax denominators l for free; l rows are
      collected into lt[32, 512] and O^T stored unnormalized.
  One batched reciprocal rt = 1/lt (a [1,512]-shaped reciprocal costs
  4us on DVE; the batched [32,512] one costs the same 4us total).
  Normalize: gpsimd partition_broadcast of rt rows + DVE in-place mult.
  C   y = O @ wo               (ACT drains psum -> yt, DMA out)
"""
import numpy as np
import ml_dtypes

B, V, S, D = 2, 8, 256, 2048
NH, NKV, KVR = 32, 8, 2
HD = D // NH  # 64
G = NH // NKV  # 4
N_CORES = 8
P = 2  # instances per core
SCALE = 1.0 / np.sqrt(HD)
BFNP = ml_dtypes.bfloat16
V0S = (0, 1, 4, 5)  # per-core first view; pair is (v0, v0+2)

_CACHE = {}


def _build():
    import concourse.tile as tile
    import concourse.mybir as mybir
    from concourse import bacc
    from contextlib import ExitStack

    F32 = mybir.dt.float32
    BF16 = mybir.dt.bfloat16
    Exp = mybir.ActivationFunctionType.Exp
    Mult = mybir.AluOpType.mult

    nc = bacc.Bacc("TRN2", target_bir_lowering=False, debug=False,
                   num_devices=N_CORES)
    xqT = nc.dram_tensor("xqT", [D, P * S], BF16, kind="ExternalInput").ap()
    xkvT = nc.dram_tensor("xkvT", [D, 768], BF16, kind="ExternalInput").ap()
    wq = nc.dram_tensor("wq", [D, D], BF16, kind="ExternalInput").ap()
    wkv = nc.dram_tensor("wkv", [D, 1024], BF16, kind="ExternalInput").ap()
    wo = nc.dram_tensor("wo", [D, D], BF16, kind="ExternalInput").ap()
    y = nc.dram_tensor("y", [P * S, D], F32, kind="ExternalOutput").ap()

    with tile.TileContext(nc) as tc, ExitStack() as top:
        xp = top.enter_context(tc.tile_pool(name="xp", bufs=1))
        wqp = top.enter_context(tc.tile_pool(name="wqp", bufs=1))
        kvp = top.enter_context(tc.tile_pool(name="kvp", bufs=1))
        qtp = top.enter_context(tc.tile_pool(name="qtp", bufs=3))
        otp = top.enter_context(tc.tile_pool(name="otp", bufs=1))
        esp = top.enter_context(tc.tile_pool(name="esp", bufs=10))
        wsp = top.enter_context(tc.tile_pool(name="wsp", bufs=6))
        wop = top.enter_context(tc.tile_pool(name="wop", bufs=6))
        ytp = top.enter_context(tc.tile_pool(name="ytp", bufs=2))
        msc = top.enter_context(tc.tile_pool(name="msc", bufs=1))
        rsp = top.enter_context(tc.tile_pool(name="rsp", bufs=3))
        pp = top.enter_context(tc.tile_pool(name="pp", bufs=8, space="PSUM"))

        # ---- input DMAs ----
        # sync queue: xkv/wk interleaved so A1 starts ~1us in; then wv.
        # gpsimd queue: xq then wq (needed only from A3 on).
        xkv = [xp.tile([128, 768], BF16, tag=f"xkv{k}", name=f"xkv{k}")
               for k in range(16)]
        wkts = []
        for k in range(16):
            nc.sync.dma_start(xkv[k][:], xkvT[k * 128:(k + 1) * 128, :])
            wkt = wsp.tile([128, 512], BF16, tag="wk", name=f"wk{k}")
            nc.scalar.dma_start(wkt[:], wkv[k * 128:(k + 1) * 128, 0:512])
            wkts.append(wkt)
        xq = [xp.tile([128, 512], BF16, tag=f"xq{k}", name=f"xq{k}")
              for k in range(16)]
        for k in range(16):
            nc.gpsimd.dma_start(xq[k][:], xqT[k * 128:(k + 1) * 128, :])
        wqsb = [wqp.tile([128, 2048], BF16, tag=f"wq{k}", name=f"wq{k}")
                for k in range(16)]
        for k in range(16):
            nc.gpsimd.dma_start(wqsb[k][:], wq[k * 128:(k + 1) * 128, :])

        # ---- persistent K^T / V_aug tiles ----
        KT = [kvp.tile([64, 768], BF16, tag=f"kt{n}", name=f"kt{n}")
              for n in range(NKV)]
        VA = [kvp.tile([128, NKV * 65], BF16, tag=f"va{t}", name=f"va{t}")
              for t in range(6)]
        for t6 in range(6):
            oc = VA[t6].rearrange("q (h c) -> q h c", c=65)[:, :, 64:65]
            nc.vector.memset(oc, 1.0)


        # ---- A1: K^T [feat, t] ----
        # NOTE: wkt slot rotation is 4-deep ("wk" tag, wsp bufs=4), but all
        # 16 DMAs were issued above; the A1 k-loop below reads them in
        # order, which the rotation already serializes correctly since the
        # DMA for slot k+4 waits on the matmuls of slot k.
        kps = [pp.tile([128, 512], F32, tag="pb", name=f"kps{i}")
               for i in range(8)]
        for k in range(16):
            for f in range(4):
                for h in range(2):
                    nc.tensor.matmul(kps[f * 2 + h][:, 0:384],
                                     wkts[k][:, f * 128:(f + 1) * 128],
                                     xkv[k][:, h * 384:(h + 1) * 384],
                                     start=(k == 0), stop=(k == 15))
        for f in range(4):
            for h in range(2):
                ps = kps[f * 2 + h]
                nc.scalar.copy(KT[2 * f][0:64, h * 384:(h + 1) * 384],
                               ps[0:64, 0:384])
                nc.vector.tensor_copy(
                    KT[2 * f + 1][0:64, h * 384:(h + 1) * 384],
                    ps[64:128, 0:384])

        # ---- A2: V natural [t, feat] + ones ----
        vps = [pp.tile([128, 512], F32, tag="pb", name=f"vps{t}")
               for t in range(6)]
        for k in range(16):
            wvt = wsp.tile([128, 512], BF16, tag="wv", name=f"wv{k}")
            nc.scalar.dma_start(wvt[:], wkv[k * 128:(k + 1) * 128, 512:1024])
            for t6 in range(6):
                nc.tensor.matmul(vps[t6][:],
                                 xkv[k][:, t6 * 128:(t6 + 1) * 128],
                                 wvt[:],
                                 start=(k == 0), stop=(k == 15))
        for t6 in range(6):
            dst = VA[t6].rearrange("q (h c) -> q h c", c=65)[:, :, 0:64]
            src = vps[t6].rearrange("q (h c) -> q h c", c=64)
            nc.vector.tensor_copy(dst, src)

        # ---- A3 + B software pipeline ----
        OT = [[otp.tile([128, 256], BF16, tag=f"ot{p}_{j}", name=f"ot{p}_{j}")
               for j in range(16)] for p in range(P)]
        # l / 1/l staging: batch b of 8 groups lives at partitions
        # 32b..32b+7 so each batch slice is quadrant-aligned.
        lt = msc.tile([128, 512], BF16, tag="lt", name="lt")
        rt32 = msc.tile([128, 512], F32, tag="rt32", name="rt32")
        rt = msc.tile([128, 512], BF16, tag="rt", name="rt")
        lrp = top.enter_context(tc.tile_pool(name="lrp", bufs=4))
        QTt = {}
        qk_ps = {}

        def emit_a3(j):
            ps = pp.tile([128, 512], F32, tag="pb", name=f"qps{j}")
            for k in range(16):
                nc.tensor.matmul(ps[:],
                                 wqsb[k][:, j * 128:(j + 1) * 128],
                                 xq[k][:],
                                 start=(k == 0), stop=(k == 15))
            qt = qtp.tile([64, 1024], BF16, tag="qt", name=f"qt{j}")
            # qt col layout: p*512 + u*256 + s  (u = head within pair)
            dst = qt[0:64].rearrange("a (p u s) -> a p u s", p=2, u=2)
            nc.vector.tensor_copy(
                dst[:, :, 0, :], ps[0:64].rearrange("a (p s) -> a p s", p=2))
            nc.vector.tensor_copy(
                dst[:, :, 1, :], ps[64:128].rearrange("a (p s) -> a p s", p=2))
            QTt[j] = qt

        def emit_b_qk(j):
            n = j // 2
            for p in range(P):
                g = j * 2 + p
                for tt in range(4):
                    qk = pp.tile([128, 512], F32, tag="pb",
                                 name=f"qk{g}_{tt}")
                    tcol = p * 256 + tt * 128
                    nc.tensor.matmul(qk[:],
                                     KT[n][0:64, tcol:tcol + 128],
                                     QTt[j][0:64, p * 512:(p + 1) * 512],
                                     start=True, stop=True)
                    e = esp.tile([128, 512], BF16, tag="e", name=f"e{g}_{tt}")
                    nc.scalar.activation(e[:], qk[:], Exp, scale=float(SCALE))
                    qk_ps[(g, tt)] = e

        def emit_b_pv(j):
            n = j // 2
            for p in range(P):
                g = j * 2 + p
                pv = pp.tile([128, 512], F32, tag="pb", name=f"pv{g}")
                for tt in range(4):
                    nc.tensor.matmul(pv[0:65, :],
                                     VA[p * 2 + tt][:, n * 65:(n + 1) * 65],
                                     qk_ps.pop((g, tt))[:],
                                     start=(tt == 0), stop=(tt == 3))
                # l row: engine APs must start at partition 0/32/64/96, so
                # stage at partition 0 then DMA-scatter into lt at partition
                # 32*(g//8) + g%8 (keeps each 8-group batch quadrant-aligned
                # for the batched reciprocal).
                lrow = lrp.tile([1, 512], BF16, tag="lr", name=f"lr{g}")
                with nc.allow_low_precision(reason="l sums in bf16"):
                    nc.vector.tensor_copy(lrow[:], pv[64:65, 0:512])
                qp = 32 * (g // 8) + (g % 8)
                nc.gpsimd.dma_start(lt[qp:qp + 1, :], lrow[:])
                nc.vector.tensor_copy(OT[p][j][0:64, :], pv[0:64, 0:256])
                nc.vector.tensor_copy(OT[p][j][64:128, :], pv[0:64, 256:512])

        rrows = {}

        def emit_recip_batch(bidx):
            # 1/l for groups 8b..8b+7 in one shot: HW reciprocal cost is
            # free-size * ~8 cycles regardless of partition count, so a
            # [8,512] batch costs the same ~4us as a single [1,512] row.
            base = 32 * bidx
            nc.vector.reciprocal(rt32[base:base + 8, :], lt[base:base + 8, :])
            with nc.allow_low_precision(reason="softmax denom in bf16"):
                nc.vector.tensor_copy(rt[base:base + 8, :],
                                      rt32[base:base + 8, :])
            # pre-issue the row gathers so emit_norm never waits on DMA
            for g in range(8 * bidx, 8 * bidx + 8):
                qp = 32 * (g // 8) + (g % 8)
                rrow = lrp.tile([1, 512], BF16, tag="rr", name=f"rr{g}",
                                bufs=9)
                nc.gpsimd.dma_start(rrow[:], rt[qp:qp + 1, :])
                rrows[g] = rrow

        def emit_norm(g):
            j, p = g // 2, g % 2
            # partition_broadcast only works with dst base partition 0 and
            # src column offset 0 (HW ucode quirk): broadcast the full row
            # on the idle Pool engine, two half-multiplies pick windows.
            rsb = rsp.tile([128, 512], BF16, tag="rsb", name=f"rsb{g}")
            nc.gpsimd.partition_broadcast(rsb[:], rrows.pop(g)[:],
                                          channels=128)
            nc.vector.tensor_tensor(OT[p][j][0:64, :], OT[p][j][0:64, :],
                                    rsb[0:64, 0:256], Mult)
            nc.gpsimd.tensor_tensor(OT[p][j][64:128, :], OT[p][j][64:128, :],
                                    rsb[64:128, 256:512], Mult)

        pending = []
        recip_todo = []
        for j in range(17):
            # the ~4.3us DVE reciprocal goes at iteration start so it only
            # delays this iteration's qt copy (needed one step later), not
            # the pv drains.
            while recip_todo:
                bidx = recip_todo.pop(0)
                emit_recip_batch(bidx)
                pending.extend(range(8 * bidx, 8 * bidx + 8))
            if j >= 1:
                emit_b_qk(j - 1)
            if j < 16:
                emit_a3(j)
            if j >= 1:
                emit_b_pv(j - 1)
                if (j - 1) % 4 == 3:
                    recip_todo.append((j - 1) // 4)
            for _ in range(2):
                if pending:
                    emit_norm(pending.pop(0))
        while recip_todo:
            bidx = recip_todo.pop(0)
            emit_recip_batch(bidx)
            pending.extend(range(8 * bidx, 8 * bidx + 8))
        while pending:
            emit_norm(pending.pop(0))

        # ---- C: y = O @ wo  (nn pairs, 8 psum banks, [128,1024] wo tiles) ----
        for half in range(2):
            acc = [pp.tile([128, 512], F32, tag="pb", name=f"acc{half}_{i}")
                   for i in range(8)]
            for k in range(16):
                wot = wop.tile([128, 1024], BF16, tag="wo",
                               name=f"wo{half}_{k}")
                nc.sync.dma_start(
                    wot[:],
                    wo[k * 128:(k + 1) * 128, half * 1024:(half + 1) * 1024])
                for n2 in range(2):
                    for p in range(P):
                        for m in range(2):
                            nc.tensor.matmul(
                                acc[n2 * 4 + p * 2 + m][:],
                                OT[p][k][:, m * 128:(m + 1) * 128],
                                wot[:, n2 * 512:(n2 + 1) * 512],
                                start=(k == 0), stop=(k == 15))
            for i, (p, m) in enumerate([(0, 0), (0, 1), (1, 0), (1, 1)]):
                yt = ytp.tile([128, 1024], F32, tag="yt",
                              name=f"yt{half}_{p}_{m}")
                for n2 in range(2):
                    nc.vector.tensor_copy(yt[:, n2 * 512:(n2 + 1) * 512],
                                          acc[n2 * 4 + p * 2 + m][:])
                r0 = p * 256 + m * 128
                q = (nc.sync, nc.gpsimd, nc.scalar, nc.sync)[i]
                q.dma_start(
                    y[r0:r0 + 128, half * 1024:(half + 1) * 1024], yt[:])

    nc.compile()
    return nc


def _get_nc():
    if "nc" not in _CACHE:
        _CACHE["nc"] = _build()
    return _CACHE["nc"]


def make_in_maps(x, wq, wkv, wo):
    x = np.asarray(x, dtype=np.float32)
    wq_b = np.asarray(wq, dtype=BFNP)
    wkv_b = np.asarray(wkv, dtype=BFNP)
    wo_b = np.asarray(wo, dtype=BFNP)
    in_maps = []
    for c in range(N_CORES):
        b, v0 = c // 4, V0S[c % 4]
        xq_c = np.ascontiguousarray(
            np.concatenate([x[b, v0].T, x[b, v0 + 2].T], axis=1)).astype(BFNP)
        xkv_c = np.ascontiguousarray(np.concatenate(
            [x[b, (v0 - 1) % V].T, x[b, (v0 + 1) % V].T,
             x[b, (v0 + 3) % V].T], axis=1)).astype(BFNP)
        in_maps.append({
            "xqT": xq_c, "xkvT": xkv_c,
            "wq": wq_b, "wkv": wkv_b, "wo": wo_b,
        })
    return in_maps


def kernel(x, wq, wkv, wo):
    from concourse.bass_utils import run_bass_kernel_spmd

    nc = _get_nc()
    in_maps = make_in_maps(x, wq, wkv, wo)
    res = run_bass_kernel_spmd(nc, in_maps, list(range(N_CORES)),
                               trace=False)
    out = np.empty((B, V, S, D), np.float32)
    for c in range(N_CORES):
        yc = res.results[c]["y"]
        b, v0 = c // 4, V0S[c % 4]
        out[b, v0] = yc[0:S]
        out[b, v0 + 2] = yc[S:2 * S]
    return out


# revision 22
# speedup vs baseline: 1.0963x; 1.0963x over previous
"""CrossViewAttention Trainium2 kernel (v7).

Shards the B*V=16 (batch, view) attention instances across 8 NeuronCores,
2 per core, paired as (b, v) and (b, v+2) so the two instances share KV
source view v+1 (each instance attends over views v-1, v+1 circular).
Per core the 3 distinct KV source views are projected once (25% fewer
K/V projection FLOPs). All matmul operands are bf16 (fp32 PSUM).

Schedule (single rotating 8-bank PSUM pool, no phase barriers):
  A1  K^T = wk^T @ x_kv^T   [feat, t]   (drain split ACT+DVE -> KT bf16)
  A2  V   = x_kv @ wv       [t, feat]   (+ ones col -> VA, DVE drain)
  A3+B software pipeline per head-pair j (lag 1):
      emit QK(j-1) -> emit A3(j) -> emit PV(j-1)
      so the ACT-engine exp of step j-1 hides under A3(j) matmuls.
  Softmax denominators l come free from PV's ones column. Per group the
  l row is staged at partition 0 (DVE), DMA-scattered into an
  [8-row x batch] quadrant-aligned stack, and each batch of 8 groups
  gets ONE [8,512] DVE reciprocal (HW reciprocal costs free-size*8cyc
  regardless of partition count). The 1/l rows are column-folded back
  to partition 0 by one DMA per batch; normalization (gpsimd
  partition_broadcast + in-place multiplies, all on the otherwise-idle
  Pool engine) is paced 2 groups per pipeline step so it never gates
  phase C. partition_broadcast HW quirk: dst/src partition base must
  be 0 (src column offsets are fine) - hence the fold to partition 0.
  C   y = O @ wo  (nn pairs, 8 psum banks, [128,1024] wo tiles; yt
      drains on DVE, y DMAs spread across queues)
"""
import numpy as np
import ml_dtypes

B, V, S, D = 2, 8, 256, 2048
NH, NKV, KVR = 32, 8, 2
HD = D // NH  # 64
G = NH // NKV  # 4
N_CORES = 8
P = 2  # instances per core
SCALE = 1.0 / np.sqrt(HD)
BFNP = ml_dtypes.bfloat16
V0S = (0, 1, 4, 5)  # per-core first view; pair is (v0, v0+2)

_CACHE = {}


def _build():
    import concourse.tile as tile
    import concourse.mybir as mybir
    from concourse import bacc
    from contextlib import ExitStack

    F32 = mybir.dt.float32
    BF16 = mybir.dt.bfloat16
    Exp = mybir.ActivationFunctionType.Exp
    Mult = mybir.AluOpType.mult

    nc = bacc.Bacc("TRN2", target_bir_lowering=False, debug=False,
                   num_devices=N_CORES)
    xqT = nc.dram_tensor("xqT", [D, P * S], BF16, kind="ExternalInput").ap()
    xkvT = nc.dram_tensor("xkvT", [D, 768], BF16, kind="ExternalInput").ap()
    wq = nc.dram_tensor("wq", [D, D], BF16, kind="ExternalInput").ap()
    wkv = nc.dram_tensor("wkv", [D, 1024], BF16, kind="ExternalInput").ap()
    wo = nc.dram_tensor("wo", [D, D], BF16, kind="ExternalInput").ap()
    y = nc.dram_tensor("y", [P * S, D], F32, kind="ExternalOutput").ap()

    with tile.TileContext(nc) as tc, ExitStack() as top:
        xp = top.enter_context(tc.tile_pool(name="xp", bufs=1))
        wqp = top.enter_context(tc.tile_pool(name="wqp", bufs=1))
        kvp = top.enter_context(tc.tile_pool(name="kvp", bufs=1))
        qtp = top.enter_context(tc.tile_pool(name="qtp", bufs=3))
        otp = top.enter_context(tc.tile_pool(name="otp", bufs=1))
        esp = top.enter_context(tc.tile_pool(name="esp", bufs=8))
        wsp = top.enter_context(tc.tile_pool(name="wsp", bufs=6))
        wop = top.enter_context(tc.tile_pool(name="wop", bufs=5))
        ytp = top.enter_context(tc.tile_pool(name="ytp", bufs=2))
        msc = top.enter_context(tc.tile_pool(name="msc", bufs=1))
        rsp = top.enter_context(tc.tile_pool(name="rsp", bufs=3))
        lrp = top.enter_context(tc.tile_pool(name="lrp", bufs=3))
        pp = top.enter_context(tc.tile_pool(name="pp", bufs=8, space="PSUM"))

        # ---- input DMAs ----
        # sync: xkv then (inside A2) wv.  scalar: wk.  gpsimd: xq then wq.
        # First A1 matmul needs only xkv[0] (sync) + wk[0] (scalar), which
        # issue in parallel on different queues.
        xkv = [xp.tile([128, 768], BF16, tag=f"xkv{k}", name=f"xkv{k}")
               for k in range(16)]
        wkts = []
        for k in range(16):
            nc.sync.dma_start(xkv[k][:], xkvT[k * 128:(k + 1) * 128, :])
            wkt = wsp.tile([128, 512], BF16, tag="wk", name=f"wk{k}")
            nc.scalar.dma_start(wkt[:], wkv[k * 128:(k + 1) * 128, 0:512])
            wkts.append(wkt)
        xq = [xp.tile([128, 512], BF16, tag=f"xq{k}", name=f"xq{k}")
              for k in range(16)]
        for k in range(16):
            nc.gpsimd.dma_start(xq[k][:], xqT[k * 128:(k + 1) * 128, :])
        wqsb = [wqp.tile([128, 2048], BF16, tag=f"wq{k}", name=f"wq{k}")
                for k in range(16)]
        for k in range(16):
            nc.gpsimd.dma_start(wqsb[k][:], wq[k * 128:(k + 1) * 128, :])

        # ---- persistent K^T / V_aug tiles ----
        KT = [kvp.tile([64, 768], BF16, tag=f"kt{n}", name=f"kt{n}")
              for n in range(NKV)]
        VA = [kvp.tile([128, NKV * 65], BF16, tag=f"va{t}", name=f"va{t}")
              for t in range(6)]
        for t6 in range(6):
            oc = VA[t6].rearrange("q (h c) -> q h c", c=65)[:, :, 64:65]
            nc.vector.memset(oc, 1.0)

        # ---- A1: K^T [feat, t] ----
        kps = [pp.tile([128, 512], F32, tag="pb", name=f"kps{i}")
               for i in range(8)]
        for k in range(16):
            for f in range(4):
                for h in range(2):
                    nc.tensor.matmul(kps[f * 2 + h][:, 0:384],
                                     wkts[k][:, f * 128:(f + 1) * 128],
                                     xkv[k][:, h * 384:(h + 1) * 384],
                                     start=(k == 0), stop=(k == 15))
        for f in range(4):
            for h in range(2):
                ps = kps[f * 2 + h]
                nc.scalar.copy(KT[2 * f][0:64, h * 384:(h + 1) * 384],
                               ps[0:64, 0:384])
                nc.vector.tensor_copy(
                    KT[2 * f + 1][0:64, h * 384:(h + 1) * 384],
                    ps[64:128, 0:384])

        # ---- A2: V natural [t, feat] + ones ----
        vps = [pp.tile([128, 512], F32, tag="pb", name=f"vps{t}")
               for t in range(6)]
        for k in range(16):
            wvt = wsp.tile([128, 512], BF16, tag="wv", name=f"wv{k}")
            nc.sync.dma_start(wvt[:], wkv[k * 128:(k + 1) * 128, 512:1024])
            for t6 in range(6):
                nc.tensor.matmul(vps[t6][:],
                                 xkv[k][:, t6 * 128:(t6 + 1) * 128],
                                 wvt[:],
                                 start=(k == 0), stop=(k == 15))
        for t6 in range(6):
            dst = VA[t6].rearrange("q (h c) -> q h c", c=65)[:, :, 0:64]
            src = vps[t6].rearrange("q (h c) -> q h c", c=64)
            nc.vector.tensor_copy(dst, src)

        # ---- A3 + B software pipeline ----
        OT = [[otp.tile([128, 256], BF16, tag=f"ot{p}_{j}", name=f"ot{p}_{j}")
               for j in range(16)] for p in range(P)]
        # l rows: batch b of 8 groups lives at partitions 32b..32b+7 so the
        # batched reciprocal slice is quadrant-aligned.
        lt = msc.tile([128, 512], BF16, tag="lt", name="lt")
        rt32 = msc.tile([128, 512], F32, tag="rt32", name="rt32")
        rt = msc.tile([128, 512], BF16, tag="rt", name="rt")
        # 1/l rows column-folded to partition 0: group 8b+i at cols i*512.
        rtc = msc.tile([1, 4096], BF16, tag="rtc", name="rtc")
        QTt = {}
        qk_es = {}

        def emit_a3(j):
            ps = pp.tile([128, 512], F32, tag="pb", name=f"qps{j}")
            for k in range(16):
                nc.tensor.matmul(ps[:],
                                 wqsb[k][:, j * 128:(j + 1) * 128],
                                 xq[k][:],
                                 start=(k == 0), stop=(k == 15))
            qt = qtp.tile([64, 1024], BF16, tag="qt", name=f"qt{j}")
            # qt col layout: p*512 + u*256 + s  (u = head within pair)
            dst = qt[0:64].rearrange("a (p u s) -> a p u s", p=2, u=2)
            nc.vector.tensor_copy(
                dst[:, :, 0, :], ps[0:64].rearrange("a (p s) -> a p s", p=2))
            nc.vector.tensor_copy(
                dst[:, :, 1, :], ps[64:128].rearrange("a (p s) -> a p s", p=2))
            QTt[j] = qt

        def emit_b_qk(j):
            n = j // 2
            for p in range(P):
                g = j * 2 + p
                for tt in range(4):
                    qk = pp.tile([128, 512], F32, tag="pb",
                                 name=f"qk{g}_{tt}")
                    tcol = p * 256 + tt * 128
                    nc.tensor.matmul(qk[:],
                                     KT[n][0:64, tcol:tcol + 128],
                                     QTt[j][0:64, p * 512:(p + 1) * 512],
                                     start=True, stop=True)
                    e = esp.tile([128, 512], BF16, tag="e", name=f"e{g}_{tt}")
                    nc.scalar.activation(e[:], qk[:], Exp, scale=float(SCALE))
                    qk_es[(g, tt)] = e

        def emit_b_pv(j):
            n = j // 2
            for p in range(P):
                g = j * 2 + p
                pv = pp.tile([128, 512], F32, tag="pb", name=f"pv{g}")
                for tt in range(4):
                    nc.tensor.matmul(pv[0:65, :],
                                     VA[p * 2 + tt][:, n * 65:(n + 1) * 65],
                                     qk_es.pop((g, tt))[:],
                                     start=(tt == 0), stop=(tt == 3))
                lrow = lrp.tile([1, 512], BF16, tag="lr", name=f"lr{g}")
                with nc.allow_low_precision(reason="l sums in bf16"):
                    nc.vector.tensor_copy(lrow[:], pv[64:65, 0:512])
                qp = 32 * (g // 8) + (g % 8)
                nc.gpsimd.dma_start(lt[qp:qp + 1, :], lrow[:])
                nc.vector.tensor_copy(OT[p][j][0:64, :], pv[0:64, 0:256])
                nc.vector.tensor_copy(OT[p][j][64:128, :], pv[0:64, 256:512])

        def emit_recip_batch(bidx):
            base = 32 * bidx
            nc.vector.reciprocal(rt32[base:base + 8, :], lt[base:base + 8, :])
            with nc.allow_low_precision(reason="softmax denom in bf16"):
                nc.vector.tensor_copy(rt[base:base + 8, :],
                                      rt32[base:base + 8, :])
            # fold the 8 rows into columns of partition 0 (one DMA)
            nc.sync.dma_start(rtc[0:1, :], rt[base:base + 8, :])

        def emit_norm(g, tail=False):
            j, p = g // 2, g % 2
            i = g % 8
            rsb = rsp.tile([128, 512], BF16, tag="rsb", name=f"rsb{g}")
            nc.gpsimd.partition_broadcast(
                rsb[:], rtc[0:1, i * 512:(i + 1) * 512], channels=128)
            eng = nc.vector if tail else nc.gpsimd
            eng.tensor_tensor(OT[p][j][0:64, :], OT[p][j][0:64, :],
                              rsb[0:64, 0:256], Mult)
            eng.tensor_tensor(OT[p][j][64:128, :], OT[p][j][64:128, :],
                              rsb[64:128, 256:512], Mult)

        pending = []
        for j in range(17):
            if j >= 1:
                emit_b_qk(j - 1)
            if j < 16:
                emit_a3(j)
            if j >= 1:
                emit_b_pv(j - 1)
            # drain norms BEFORE a new recip/fold: the fold overwrites rtc,
            # so all reads of the previous batch must already be emitted
            # (the WAR dep then orders the fold after them).
            for _ in range(2):
                if pending:
                    emit_norm(pending.pop(0))
            if j >= 1 and (j - 1) % 4 == 3:
                bidx = (j - 1) // 4
                emit_recip_batch(bidx)
                pending.extend(range(8 * bidx, 8 * bidx + 8))
        # tail: broadcasts stay on Pool, multiplies go to the now-idle DVE
        while pending:
            emit_norm(pending.pop(0), tail=True)

        # ---- C: y = O @ wo  (nn pairs, 8 psum banks, [128,1024] wo) ----
        for half in range(2):
            acc = [pp.tile([128, 512], F32, tag="pb", name=f"acc{half}_{i}")
                   for i in range(8)]
            for k in range(16):
                wot = wop.tile([128, 1024], BF16, tag="wo",
                               name=f"wo{half}_{k}")
                nc.sync.dma_start(
                    wot[:],
                    wo[k * 128:(k + 1) * 128, half * 1024:(half + 1) * 1024])
                for n2 in range(2):
                    for p in range(P):
                        for m in range(2):
                            nc.tensor.matmul(
                                acc[n2 * 4 + p * 2 + m][:],
                                OT[p][k][:, m * 128:(m + 1) * 128],
                                wot[:, n2 * 512:(n2 + 1) * 512],
                                start=(k == 0), stop=(k == 15))
            for i, (p, m) in enumerate([(0, 0), (0, 1), (1, 0), (1, 1)]):
                yt = ytp.tile([128, 1024], F32, tag="yt",
                              name=f"yt{half}_{p}_{m}")
                for n2 in range(2):
                    nc.vector.tensor_copy(yt[:, n2 * 512:(n2 + 1) * 512],
                                          acc[n2 * 4 + p * 2 + m][:])
                r0 = p * 256 + m * 128
                q = (nc.sync, nc.gpsimd, nc.scalar, nc.sync)[i]
                q.dma_start(
                    y[r0:r0 + 128, half * 1024:(half + 1) * 1024], yt[:])

    nc.compile()
    return nc


def _get_nc():
    if "nc" not in _CACHE:
        _CACHE["nc"] = _build()
    return _CACHE["nc"]


def make_in_maps(x, wq, wkv, wo):
    x = np.asarray(x, dtype=np.float32)
    wq_b = np.asarray(wq, dtype=BFNP)
    wkv_b = np.asarray(wkv, dtype=BFNP)
    wo_b = np.asarray(wo, dtype=BFNP)
    in_maps = []
    for c in range(N_CORES):
        b, v0 = c // 4, V0S[c % 4]
        xq_c = np.ascontiguousarray(
            np.concatenate([x[b, v0].T, x[b, v0 + 2].T], axis=1)).astype(BFNP)
        xkv_c = np.ascontiguousarray(np.concatenate(
            [x[b, (v0 - 1) % V].T, x[b, (v0 + 1) % V].T,
             x[b, (v0 + 3) % V].T], axis=1)).astype(BFNP)
        in_maps.append({
            "xqT": xq_c, "xkvT": xkv_c,
            "wq": wq_b, "wkv": wkv_b, "wo": wo_b,
        })
    return in_maps


def kernel(x, wq, wkv, wo):
    from concourse.bass_utils import run_bass_kernel_spmd

    nc = _get_nc()
    in_maps = make_in_maps(x, wq, wkv, wo)
    res = run_bass_kernel_spmd(nc, in_maps, list(range(N_CORES)),
                               trace=False)
    out = np.empty((B, V, S, D), np.float32)
    for c in range(N_CORES):
        yc = res.results[c]["y"]
        b, v0 = c // 4, V0S[c % 4]
        out[b, v0] = yc[0:S]
        out[b, v0 + 2] = yc[S:2 * S]
    return out


# revision 23
# speedup vs baseline: 1.5665x; 1.4290x over previous
"""CrossViewAttention Trainium2 kernel (v7).

Shards the B*V=16 (batch, view) attention instances across 8 NeuronCores,
2 per core, paired as (b, v) and (b, v+2) so the two instances share KV
source view v+1 (each instance attends over views v-1, v+1 circular).
Per core the 3 distinct KV source views are projected once (25% fewer
K/V projection FLOPs). All matmul operands are bf16 (fp32 PSUM).

Schedule (single rotating 8-bank PSUM pool, no phase barriers):
  A1  K^T = wk^T @ x_kv^T   [feat, t]   (drain split ACT+DVE -> KT bf16)
  A2  V   = x_kv @ wv       [t, feat]   (+ ones col -> VA, DVE drain)
  A3+B software pipeline per head-pair j (lag 1):
      emit QK(j-1) -> emit A3(j) -> emit PV(j-1)
      so the ACT-engine exp of step j-1 hides under A3(j) matmuls.
  Softmax denominators l come free from PV's ones column. Per group the
  l row is staged at partition 0 (DVE), DMA-scattered into an
  [8-row x batch] quadrant-aligned stack, and each batch of 8 groups
  gets ONE [8,512] DVE reciprocal (HW reciprocal costs free-size*8cyc
  regardless of partition count). The 1/l rows are column-folded back
  to partition 0 by one DMA per batch; normalization (gpsimd
  partition_broadcast + in-place multiplies, all on the otherwise-idle
  Pool engine) is paced 2 groups per pipeline step so it never gates
  phase C. partition_broadcast HW quirk: dst/src partition base must
  be 0 (src column offsets are fine) - hence the fold to partition 0.
  C   y = O @ wo  (nn pairs, 8 psum banks, [128,1024] wo tiles; yt
      drains on DVE, y DMAs spread across queues)
"""
import numpy as np
import ml_dtypes

B, V, S, D = 2, 8, 256, 2048
NH, NKV, KVR = 32, 8, 2
HD = D // NH  # 64
G = NH // NKV  # 4
N_CORES = 8
P = 2  # instances per core
SCALE = 1.0 / np.sqrt(HD)
BFNP = ml_dtypes.bfloat16
V0S = (0, 1, 4, 5)  # per-core first view; pair is (v0, v0+2)

_CACHE = {}


def _build():
    import concourse.tile as tile
    import concourse.mybir as mybir
    from concourse import bacc
    from contextlib import ExitStack

    F32 = mybir.dt.float32
    BF16 = mybir.dt.bfloat16
    Exp = mybir.ActivationFunctionType.Exp
    Mult = mybir.AluOpType.mult

    nc = bacc.Bacc("TRN2", target_bir_lowering=False, debug=False,
                   num_devices=N_CORES)
    xqT = nc.dram_tensor("xqT", [D, P * S], BF16, kind="ExternalInput").ap()
    xkvT = nc.dram_tensor("xkvT", [D, 768], BF16, kind="ExternalInput").ap()
    wq = nc.dram_tensor("wq", [D, D], BF16, kind="ExternalInput").ap()
    wkv = nc.dram_tensor("wkv", [D, 1024], BF16, kind="ExternalInput").ap()
    wo = nc.dram_tensor("wo", [D, D], BF16, kind="ExternalInput").ap()
    y = nc.dram_tensor("y", [P * S, D], F32, kind="ExternalOutput").ap()

    with tile.TileContext(nc) as tc, ExitStack() as top:
        xp = top.enter_context(tc.tile_pool(name="xp", bufs=1))
        wqp = top.enter_context(tc.tile_pool(name="wqp", bufs=1))
        kvp = top.enter_context(tc.tile_pool(name="kvp", bufs=1))
        qtp = top.enter_context(tc.tile_pool(name="qtp", bufs=3))
        otp = top.enter_context(tc.tile_pool(name="otp", bufs=1))
        esp = top.enter_context(tc.tile_pool(name="esp", bufs=8))
        wsp = top.enter_context(tc.tile_pool(name="wsp", bufs=6))
        wop = top.enter_context(tc.tile_pool(name="wop", bufs=5))
        ytp = top.enter_context(tc.tile_pool(name="ytp", bufs=2))
        msc = top.enter_context(tc.tile_pool(name="msc", bufs=1))
        rsp = top.enter_context(tc.tile_pool(name="rsp", bufs=3))
        lrp = top.enter_context(tc.tile_pool(name="lrp", bufs=3))
        pp = top.enter_context(tc.tile_pool(name="pp", bufs=8, space="PSUM"))

        # ---- input DMAs ----
        # sync: xkv then (inside A2) wv.  scalar: wk.  gpsimd: xq then wq.
        # First A1 matmul needs only xkv[0] (sync) + wk[0] (scalar), which
        # issue in parallel on different queues.
        xkv = [xp.tile([128, 768], BF16, tag=f"xkv{k}", name=f"xkv{k}")
               for k in range(16)]
        wkts = []
        for k in range(16):
            nc.sync.dma_start(xkv[k][:], xkvT[k * 128:(k + 1) * 128, :])
            wkt = wsp.tile([128, 512], BF16, tag="wk", name=f"wk{k}")
            nc.scalar.dma_start(wkt[:], wkv[k * 128:(k + 1) * 128, 0:512])
            wkts.append(wkt)
        xq = [xp.tile([128, 512], BF16, tag=f"xq{k}", name=f"xq{k}")
              for k in range(16)]
        for k in range(16):
            nc.gpsimd.dma_start(xq[k][:], xqT[k * 128:(k + 1) * 128, :])
        wqsb = [wqp.tile([128, 2048], BF16, tag=f"wq{k}", name=f"wq{k}")
                for k in range(16)]
        for k in range(16):
            nc.gpsimd.dma_start(wqsb[k][:], wq[k * 128:(k + 1) * 128, :])

        # ---- persistent K^T / V_aug tiles ----
        KT = [kvp.tile([64, 768], BF16, tag=f"kt{n}", name=f"kt{n}")
              for n in range(NKV)]
        VA = [kvp.tile([128, NKV * 65], BF16, tag=f"va{t}", name=f"va{t}")
              for t in range(6)]
        for t6 in range(6):
            oc = VA[t6].rearrange("q (h c) -> q h c", c=65)[:, :, 64:65]
            nc.vector.memset(oc, 1.0)

        # ---- A1: K^T [feat, t] ----
        kps = [pp.tile([128, 512], F32, tag="pb", name=f"kps{i}")
               for i in range(8)]
        for k in range(16):
            for f in range(4):
                for h in range(2):
                    nc.tensor.matmul(kps[f * 2 + h][:, 0:384],
                                     wkts[k][:, f * 128:(f + 1) * 128],
                                     xkv[k][:, h * 384:(h + 1) * 384],
                                     start=(k == 0), stop=(k == 15))
        for f in range(4):
            for h in range(2):
                ps = kps[f * 2 + h]
                nc.scalar.copy(KT[2 * f][0:64, h * 384:(h + 1) * 384],
                               ps[0:64, 0:384])
                nc.vector.tensor_copy(
                    KT[2 * f + 1][0:64, h * 384:(h + 1) * 384],
                    ps[64:128, 0:384])

        # ---- A2: V natural [t, feat] + ones ----
        vps = [pp.tile([128, 512], F32, tag="pb", name=f"vps{t}")
               for t in range(6)]
        for k in range(16):
            wvt = wsp.tile([128, 512], BF16, tag="wv", name=f"wv{k}")
            nc.sync.dma_start(wvt[:], wkv[k * 128:(k + 1) * 128, 512:1024])
            for t6 in range(6):
                nc.tensor.matmul(vps[t6][:],
                                 xkv[k][:, t6 * 128:(t6 + 1) * 128],
                                 wvt[:],
                                 start=(k == 0), stop=(k == 15))
        for t6 in range(6):
            dst = VA[t6].rearrange("q (h c) -> q h c", c=65)[:, :, 0:64]
            src = vps[t6].rearrange("q (h c) -> q h c", c=64)
            nc.vector.tensor_copy(dst, src)

        # ---- A3 + B software pipeline ----
        OT = [[otp.tile([128, 256], BF16, tag=f"ot{p}_{j}", name=f"ot{p}_{j}")
               for j in range(16)] for p in range(P)]
        # l rows: batch b of 8 groups lives at partitions 32b..32b+7 so the
        # batched reciprocal slice is quadrant-aligned.
        lt = msc.tile([128, 512], BF16, tag="lt", name="lt")
        rt32 = msc.tile([128, 512], F32, tag="rt32", name="rt32")
        rt = msc.tile([128, 512], BF16, tag="rt", name="rt")
        # 1/l rows column-folded to partition 0: group 8b+i at cols i*512.
        rtc = msc.tile([1, 4096], BF16, tag="rtc", name="rtc")
        QTt = {}
        qk_es = {}

        def emit_a3(j):
            ps = pp.tile([128, 512], F32, tag="pb", name=f"qps{j}")
            for k in range(16):
                nc.tensor.matmul(ps[:],
                                 wqsb[k][:, j * 128:(j + 1) * 128],
                                 xq[k][:],
                                 start=(k == 0), stop=(k == 15))
            qt = qtp.tile([64, 1024], BF16, tag="qt", name=f"qt{j}")
            # qt col layout: p*512 + u*256 + s  (u = head within pair)
            dst = qt[0:64].rearrange("a (p u s) -> a p u s", p=2, u=2)
            nc.vector.tensor_copy(
                dst[:, :, 0, :], ps[0:64].rearrange("a (p s) -> a p s", p=2))
            nc.vector.tensor_copy(
                dst[:, :, 1, :], ps[64:128].rearrange("a (p s) -> a p s", p=2))
            QTt[j] = qt

        def emit_b_qk(j):
            n = j // 2
            for p in range(P):
                g = j * 2 + p
                for tt in range(4):
                    qk = pp.tile([128, 512], F32, tag="pb",
                                 name=f"qk{g}_{tt}")
                    tcol = p * 256 + tt * 128
                    nc.tensor.matmul(qk[:],
                                     KT[n][0:64, tcol:tcol + 128],
                                     QTt[j][0:64, p * 512:(p + 1) * 512],
                                     start=True, stop=True)
                    e = esp.tile([128, 512], BF16, tag="e", name=f"e{g}_{tt}")
                    nc.scalar.activation(e[:], qk[:], Exp, scale=float(SCALE))
                    qk_es[(g, tt)] = e

        def emit_b_pv(j):
            n = j // 2
            for p in range(P):
                g = j * 2 + p
                pv = pp.tile([128, 512], F32, tag="pb", name=f"pv{g}")
                for tt in range(4):
                    nc.tensor.matmul(pv[0:65, :],
                                     VA[p * 2 + tt][:, n * 65:(n + 1) * 65],
                                     qk_es.pop((g, tt))[:],
                                     start=(tt == 0), stop=(tt == 3))
                lrow = lrp.tile([1, 512], BF16, tag="lr", name=f"lr{g}")
                with nc.allow_low_precision(reason="l sums in bf16"):
                    nc.vector.tensor_copy(lrow[:], pv[64:65, 0:512])
                qp = 32 * (g // 8) + (g % 8)
                nc.gpsimd.dma_start(lt[qp:qp + 1, :], lrow[:])
                nc.vector.tensor_copy(OT[p][j][0:64, :], pv[0:64, 0:256])
                nc.vector.tensor_copy(OT[p][j][64:128, :], pv[0:64, 256:512])

        def emit_recip_batch(bidx):
            base = 32 * bidx
            nc.vector.reciprocal(rt32[base:base + 8, :], lt[base:base + 8, :])
            with nc.allow_low_precision(reason="softmax denom in bf16"):
                nc.vector.tensor_copy(rt[base:base + 8, :],
                                      rt32[base:base + 8, :])
            # fold the 8 rows into columns of partition 0 (one DMA)
            nc.sync.dma_start(rtc[0:1, :], rt[base:base + 8, :])

        def emit_norm(g, tail=False):
            # NOTE: the multiplies must NOT run on gpsimd - mixing standard
            # ops with the partition_broadcast ucode on the Pool engine
            # costs a ~6us pipeline switch per transition (measured).
            j, p = g // 2, g % 2
            i = g % 8
            rsb = rsp.tile([128, 512], BF16, tag="rsb", name=f"rsb{g}")
            nc.gpsimd.partition_broadcast(
                rsb[:], rtc[0:1, i * 512:(i + 1) * 512], channels=128)
            nc.vector.tensor_tensor(OT[p][j][0:64, :], OT[p][j][0:64, :],
                                    rsb[0:64, 0:256], Mult)
            nc.vector.tensor_tensor(OT[p][j][64:128, :], OT[p][j][64:128, :],
                                    rsb[64:128, 256:512], Mult)

        pending = []
        for j in range(17):
            if j >= 1:
                emit_b_qk(j - 1)
            if j < 16:
                emit_a3(j)
            if j >= 1:
                emit_b_pv(j - 1)
            # drain norms BEFORE a new recip/fold: the fold overwrites rtc,
            # so all reads of the previous batch must already be emitted
            # (the WAR dep then orders the fold after them).
            for _ in range(2):
                if pending:
                    emit_norm(pending.pop(0))
            if j >= 1 and (j - 1) % 4 == 3:
                bidx = (j - 1) // 4
                emit_recip_batch(bidx)
                pending.extend(range(8 * bidx, 8 * bidx + 8))
        # tail: broadcasts stay on Pool, multiplies go to the now-idle DVE
        while pending:
            emit_norm(pending.pop(0), tail=True)

        # ---- C: y = O @ wo  (nn pairs, 8 psum banks, [128,1024] wo) ----
        for half in range(2):
            acc = [pp.tile([128, 512], F32, tag="pb", name=f"acc{half}_{i}")
                   for i in range(8)]
            for k in range(16):
                wot = wop.tile([128, 1024], BF16, tag="wo",
                               name=f"wo{half}_{k}")
                nc.sync.dma_start(
                    wot[:],
                    wo[k * 128:(k + 1) * 128, half * 1024:(half + 1) * 1024])
                for n2 in range(2):
                    for p in range(P):
                        for m in range(2):
                            nc.tensor.matmul(
                                acc[n2 * 4 + p * 2 + m][:],
                                OT[p][k][:, m * 128:(m + 1) * 128],
                                wot[:, n2 * 512:(n2 + 1) * 512],
                                start=(k == 0), stop=(k == 15))
            for i, (p, m) in enumerate([(0, 0), (0, 1), (1, 0), (1, 1)]):
                yt = ytp.tile([128, 1024], F32, tag="yt",
                              name=f"yt{half}_{p}_{m}")
                for n2 in range(2):
                    nc.vector.tensor_copy(yt[:, n2 * 512:(n2 + 1) * 512],
                                          acc[n2 * 4 + p * 2 + m][:])
                r0 = p * 256 + m * 128
                q = (nc.sync, nc.gpsimd, nc.scalar, nc.sync)[i]
                q.dma_start(
                    y[r0:r0 + 128, half * 1024:(half + 1) * 1024], yt[:])

    nc.compile()
    return nc


def _get_nc():
    if "nc" not in _CACHE:
        _CACHE["nc"] = _build()
    return _CACHE["nc"]


def make_in_maps(x, wq, wkv, wo):
    x = np.asarray(x, dtype=np.float32)
    wq_b = np.asarray(wq, dtype=BFNP)
    wkv_b = np.asarray(wkv, dtype=BFNP)
    wo_b = np.asarray(wo, dtype=BFNP)
    in_maps = []
    for c in range(N_CORES):
        b, v0 = c // 4, V0S[c % 4]
        xq_c = np.ascontiguousarray(
            np.concatenate([x[b, v0].T, x[b, v0 + 2].T], axis=1)).astype(BFNP)
        xkv_c = np.ascontiguousarray(np.concatenate(
            [x[b, (v0 - 1) % V].T, x[b, (v0 + 1) % V].T,
             x[b, (v0 + 3) % V].T], axis=1)).astype(BFNP)
        in_maps.append({
            "xqT": xq_c, "xkvT": xkv_c,
            "wq": wq_b, "wkv": wkv_b, "wo": wo_b,
        })
    return in_maps


def kernel(x, wq, wkv, wo):
    from concourse.bass_utils import run_bass_kernel_spmd

    nc = _get_nc()
    in_maps = make_in_maps(x, wq, wkv, wo)
    res = run_bass_kernel_spmd(nc, in_maps, list(range(N_CORES)),
                               trace=False)
    out = np.empty((B, V, S, D), np.float32)
    for c in range(N_CORES):
        yc = res.results[c]["y"]
        b, v0 = c // 4, V0S[c % 4]
        out[b, v0] = yc[0:S]
        out[b, v0 + 2] = yc[S:2 * S]
    return out


# revision 24
# speedup vs baseline: 1.5818x; 1.0098x over previous
"""CrossViewAttention Trainium2 kernel (v7).

Shards the B*V=16 (batch, view) attention instances across 8 NeuronCores,
2 per core, paired as (b, v) and (b, v+2) so the two instances share KV
source view v+1 (each instance attends over views v-1, v+1 circular).
Per core the 3 distinct KV source views are projected once (25% fewer
K/V projection FLOPs). All matmul operands are bf16 (fp32 PSUM).

Schedule (single rotating 8-bank PSUM pool, no phase barriers):
  A1  K^T = wk^T @ x_kv^T   [feat, t]   (drain split ACT+DVE -> KT bf16)
  A2  V   = x_kv @ wv       [t, feat]   (+ ones col -> VA, DVE drain)
  A3+B software pipeline per head-pair j (lag 1):
      emit QK(j-1) -> emit A3(j) -> emit PV(j-1)
      so the ACT-engine exp of step j-1 hides under A3(j) matmuls.
  Softmax denominators l come free from PV's ones column. Per group the
  l row is staged at partition 0 (DVE), DMA-scattered into an
  [8-row x batch] quadrant-aligned stack, and each batch of 8 groups
  gets ONE [8,512] DVE reciprocal (HW reciprocal costs free-size*8cyc
  regardless of partition count). The 1/l rows are column-folded back
  to partition 0 by one DMA per batch; normalization (gpsimd
  partition_broadcast + in-place multiplies, all on the otherwise-idle
  Pool engine) is paced 2 groups per pipeline step so it never gates
  phase C. partition_broadcast HW quirk: dst/src partition base must
  be 0 (src column offsets are fine) - hence the fold to partition 0.
  C   y = O @ wo  (nn pairs, 8 psum banks, [128,1024] wo tiles; yt
      drains on DVE, y DMAs spread across queues)
"""
import numpy as np
import ml_dtypes

B, V, S, D = 2, 8, 256, 2048
NH, NKV, KVR = 32, 8, 2
HD = D // NH  # 64
G = NH // NKV  # 4
N_CORES = 8
P = 2  # instances per core
SCALE = 1.0 / np.sqrt(HD)
BFNP = ml_dtypes.bfloat16
V0S = (0, 1, 4, 5)  # per-core first view; pair is (v0, v0+2)

_CACHE = {}


def _build():
    import concourse.tile as tile
    import concourse.mybir as mybir
    from concourse import bacc
    from contextlib import ExitStack

    F32 = mybir.dt.float32
    BF16 = mybir.dt.bfloat16
    Exp = mybir.ActivationFunctionType.Exp
    Mult = mybir.AluOpType.mult

    nc = bacc.Bacc("TRN2", target_bir_lowering=False, debug=False,
                   num_devices=N_CORES)
    xqT = nc.dram_tensor("xqT", [D, P * S], BF16, kind="ExternalInput").ap()
    xkvT = nc.dram_tensor("xkvT", [D, 768], BF16, kind="ExternalInput").ap()
    wq = nc.dram_tensor("wq", [D, D], BF16, kind="ExternalInput").ap()
    wkv = nc.dram_tensor("wkv", [D, 1024], BF16, kind="ExternalInput").ap()
    wo = nc.dram_tensor("wo", [D, D], BF16, kind="ExternalInput").ap()
    y = nc.dram_tensor("y", [P * S, D], F32, kind="ExternalOutput").ap()

    with tile.TileContext(nc) as tc, ExitStack() as top:
        xp = top.enter_context(tc.tile_pool(name="xp", bufs=1))
        wqp = top.enter_context(tc.tile_pool(name="wqp", bufs=1))
        kvp = top.enter_context(tc.tile_pool(name="kvp", bufs=1))
        qtp = top.enter_context(tc.tile_pool(name="qtp", bufs=3))
        otp = top.enter_context(tc.tile_pool(name="otp", bufs=1))
        esp = top.enter_context(tc.tile_pool(name="esp", bufs=8))
        wsp = top.enter_context(tc.tile_pool(name="wsp", bufs=6))
        wop = top.enter_context(tc.tile_pool(name="wop", bufs=5))
        ytp = top.enter_context(tc.tile_pool(name="ytp", bufs=2))
        msc = top.enter_context(tc.tile_pool(name="msc", bufs=1))
        rsp = top.enter_context(tc.tile_pool(name="rsp", bufs=3))
        lrp = top.enter_context(tc.tile_pool(name="lrp", bufs=3))
        pp = top.enter_context(tc.tile_pool(name="pp", bufs=8, space="PSUM"))

        # ---- input DMAs ----
        # sync: xkv then (inside A2) wv.  scalar: wk.  gpsimd: xq then wq.
        # First A1 matmul needs only xkv[0] (sync) + wk[0] (scalar), which
        # issue in parallel on different queues.
        xkv = [xp.tile([128, 768], BF16, tag=f"xkv{k}", name=f"xkv{k}")
               for k in range(16)]
        wkts = []
        for k in range(16):
            nc.sync.dma_start(xkv[k][:], xkvT[k * 128:(k + 1) * 128, :])
            wkt = wsp.tile([128, 512], BF16, tag="wk", name=f"wk{k}")
            nc.scalar.dma_start(wkt[:], wkv[k * 128:(k + 1) * 128, 0:512])
            wkts.append(wkt)
        xq = [xp.tile([128, 512], BF16, tag=f"xq{k}", name=f"xq{k}")
              for k in range(16)]
        for k in range(16):
            nc.gpsimd.dma_start(xq[k][:], xqT[k * 128:(k + 1) * 128, :])
        wqsb = [wqp.tile([128, 2048], BF16, tag=f"wq{k}", name=f"wq{k}")
                for k in range(16)]
        for k in range(16):
            nc.gpsimd.dma_start(wqsb[k][:], wq[k * 128:(k + 1) * 128, :])

        # ---- persistent K^T / V_aug tiles ----
        KT = [kvp.tile([64, 768], BF16, tag=f"kt{n}", name=f"kt{n}")
              for n in range(NKV)]
        VA = [kvp.tile([128, NKV * 65], BF16, tag=f"va{t}", name=f"va{t}")
              for t in range(6)]
        for t6 in range(6):
            oc = VA[t6].rearrange("q (h c) -> q h c", c=65)[:, :, 64:65]
            nc.vector.memset(oc, 1.0)

        # ---- A1: K^T [feat, t] ----
        kps = [pp.tile([128, 512], F32, tag="pb", name=f"kps{i}")
               for i in range(8)]
        for k in range(16):
            for f in range(4):
                for h in range(2):
                    nc.tensor.matmul(kps[f * 2 + h][:, 0:384],
                                     wkts[k][:, f * 128:(f + 1) * 128],
                                     xkv[k][:, h * 384:(h + 1) * 384],
                                     start=(k == 0), stop=(k == 15))
        for f in range(4):
            for h in range(2):
                ps = kps[f * 2 + h]
                nc.scalar.copy(KT[2 * f][0:64, h * 384:(h + 1) * 384],
                               ps[0:64, 0:384])
                nc.vector.tensor_copy(
                    KT[2 * f + 1][0:64, h * 384:(h + 1) * 384],
                    ps[64:128, 0:384])

        # ---- A2: V natural [t, feat] + ones ----
        vps = [pp.tile([128, 512], F32, tag="pb", name=f"vps{t}")
               for t in range(6)]
        for k in range(16):
            wvt = wsp.tile([128, 512], BF16, tag="wv", name=f"wv{k}")
            nc.sync.dma_start(wvt[:], wkv[k * 128:(k + 1) * 128, 512:1024])
            for t6 in range(6):
                nc.tensor.matmul(vps[t6][:],
                                 xkv[k][:, t6 * 128:(t6 + 1) * 128],
                                 wvt[:],
                                 start=(k == 0), stop=(k == 15))
        for t6 in range(6):
            dst = VA[t6].rearrange("q (h c) -> q h c", c=65)[:, :, 0:64]
            src = vps[t6].rearrange("q (h c) -> q h c", c=64)
            nc.vector.tensor_copy(dst, src)

        # ---- A3 + B software pipeline ----
        OT = [[otp.tile([128, 256], BF16, tag=f"ot{p}_{j}", name=f"ot{p}_{j}")
               for j in range(16)] for p in range(P)]
        # l rows: batch b of 8 groups lives at partitions 32b..32b+7 so the
        # batched reciprocal slice is quadrant-aligned.
        lt = msc.tile([128, 512], BF16, tag="lt", name="lt")
        rt32 = msc.tile([128, 512], F32, tag="rt32", name="rt32")
        rt = msc.tile([128, 512], BF16, tag="rt", name="rt")
        # 1/l rows column-folded to partition 0: group 8b+i at cols i*512.
        rtc = msc.tile([1, 4096], BF16, tag="rtc", name="rtc")
        QTt = {}
        qk_es = {}

        def emit_a3(j):
            ps = pp.tile([128, 512], F32, tag="pb", name=f"qps{j}")
            for k in range(16):
                nc.tensor.matmul(ps[:],
                                 wqsb[k][:, j * 128:(j + 1) * 128],
                                 xq[k][:],
                                 start=(k == 0), stop=(k == 15))
            qt = qtp.tile([64, 1024], BF16, tag="qt", name=f"qt{j}")
            # qt col layout: p*512 + u*256 + s  (u = head within pair).
            # These casts run on ACT so the batched DVE reciprocal can
            # never delay them (QK of the next step needs qt).
            dst = qt[0:64].rearrange("a (p u s) -> a p u s", p=2, u=2)
            nc.scalar.copy(
                dst[:, :, 0, :], ps[0:64].rearrange("a (p s) -> a p s", p=2))
            nc.scalar.copy(
                dst[:, :, 1, :], ps[64:128].rearrange("a (p s) -> a p s", p=2))
            QTt[j] = qt

        def emit_b_qk(j):
            n = j // 2
            for p in range(P):
                g = j * 2 + p
                for tt in range(4):
                    qk = pp.tile([128, 512], F32, tag="pb",
                                 name=f"qk{g}_{tt}")
                    tcol = p * 256 + tt * 128
                    nc.tensor.matmul(qk[:],
                                     KT[n][0:64, tcol:tcol + 128],
                                     QTt[j][0:64, p * 512:(p + 1) * 512],
                                     start=True, stop=True)
                    e = esp.tile([128, 512], BF16, tag="e", name=f"e{g}_{tt}")
                    nc.scalar.activation(e[:], qk[:], Exp, scale=float(SCALE))
                    qk_es[(g, tt)] = e

        def emit_b_pv(j):
            n = j // 2
            for p in range(P):
                g = j * 2 + p
                pv = pp.tile([128, 512], F32, tag="pb", name=f"pv{g}")
                for tt in range(4):
                    nc.tensor.matmul(pv[0:65, :],
                                     VA[p * 2 + tt][:, n * 65:(n + 1) * 65],
                                     qk_es.pop((g, tt))[:],
                                     start=(tt == 0), stop=(tt == 3))
                lrow = lrp.tile([1, 512], BF16, tag="lr", name=f"lr{g}")
                with nc.allow_low_precision(reason="l sums in bf16"):
                    nc.vector.tensor_copy(lrow[:], pv[64:65, 0:512])
                qp = 32 * (g // 8) + (g % 8)
                nc.gpsimd.dma_start(lt[qp:qp + 1, :], lrow[:])
                nc.vector.tensor_copy(OT[p][j][0:64, :], pv[0:64, 0:256])
                nc.vector.tensor_copy(OT[p][j][64:128, :], pv[0:64, 256:512])

        def emit_recip_batch(bidx):
            base = 32 * bidx
            nc.vector.reciprocal(rt32[base:base + 8, :], lt[base:base + 8, :])
            with nc.allow_low_precision(reason="softmax denom in bf16"):
                nc.vector.tensor_copy(rt[base:base + 8, :],
                                      rt32[base:base + 8, :])
            # fold the 8 rows into columns of partition 0 (one DMA)
            nc.sync.dma_start(rtc[0:1, :], rt[base:base + 8, :])

        def emit_norm(g, tail=False):
            # NOTE: the multiplies must NOT run on gpsimd - mixing standard
            # ops with the partition_broadcast ucode on the Pool engine
            # costs a ~6us pipeline switch per transition (measured).
            j, p = g // 2, g % 2
            i = g % 8
            rsb = rsp.tile([128, 512], BF16, tag="rsb", name=f"rsb{g}")
            nc.gpsimd.partition_broadcast(
                rsb[:], rtc[0:1, i * 512:(i + 1) * 512], channels=128)
            nc.vector.tensor_tensor(OT[p][j][0:64, :], OT[p][j][0:64, :],
                                    rsb[0:64, 0:256], Mult)
            nc.vector.tensor_tensor(OT[p][j][64:128, :], OT[p][j][64:128, :],
                                    rsb[64:128, 256:512], Mult)

        pending = []
        for j in range(17):
            if j >= 1:
                emit_b_qk(j - 1)
            if j < 16:
                emit_a3(j)
            if j >= 1:
                emit_b_pv(j - 1)
            # drain norms BEFORE a new recip/fold: the fold overwrites rtc,
            # so all reads of the previous batch must already be emitted
            # (the WAR dep then orders the fold after them).
            for _ in range(2):
                if pending:
                    emit_norm(pending.pop(0))
            if j >= 1 and (j - 1) % 4 == 3:
                bidx = (j - 1) // 4
                emit_recip_batch(bidx)
                pending.extend(range(8 * bidx, 8 * bidx + 8))
        # tail: broadcasts stay on Pool, multiplies go to the now-idle DVE
        while pending:
            emit_norm(pending.pop(0), tail=True)

        # ---- C: y = O @ wo  (nn pairs, 8 psum banks, [128,1024] wo) ----
        for half in range(2):
            acc = [pp.tile([128, 512], F32, tag="pb", name=f"acc{half}_{i}")
                   for i in range(8)]
            for k in range(16):
                wot = wop.tile([128, 1024], BF16, tag="wo",
                               name=f"wo{half}_{k}")
                nc.sync.dma_start(
                    wot[:],
                    wo[k * 128:(k + 1) * 128, half * 1024:(half + 1) * 1024])
                for n2 in range(2):
                    for p in range(P):
                        for m in range(2):
                            nc.tensor.matmul(
                                acc[n2 * 4 + p * 2 + m][:],
                                OT[p][k][:, m * 128:(m + 1) * 128],
                                wot[:, n2 * 512:(n2 + 1) * 512],
                                start=(k == 0), stop=(k == 15))
            for i, (p, m) in enumerate([(0, 0), (0, 1), (1, 0), (1, 1)]):
                yt = ytp.tile([128, 1024], F32, tag="yt",
                              name=f"yt{half}_{p}_{m}")
                for n2 in range(2):
                    eng = nc.vector if (i + n2) % 2 == 0 else nc.scalar
                    if eng is nc.vector:
                        eng.tensor_copy(yt[:, n2 * 512:(n2 + 1) * 512],
                                        acc[n2 * 4 + p * 2 + m][:])
                    else:
                        eng.copy(yt[:, n2 * 512:(n2 + 1) * 512],
                                 acc[n2 * 4 + p * 2 + m][:])
                r0 = p * 256 + m * 128
                q = (nc.sync, nc.gpsimd, nc.scalar, nc.sync)[i]
                q.dma_start(
                    y[r0:r0 + 128, half * 1024:(half + 1) * 1024], yt[:])

    nc.compile()
    return nc


def _get_nc():
    if "nc" not in _CACHE:
        _CACHE["nc"] = _build()
    return _CACHE["nc"]


def make_in_maps(x, wq, wkv, wo):
    x = np.asarray(x, dtype=np.float32)
    wq_b = np.asarray(wq, dtype=BFNP)
    wkv_b = np.asarray(wkv, dtype=BFNP)
    wo_b = np.asarray(wo, dtype=BFNP)
    in_maps = []
    for c in range(N_CORES):
        b, v0 = c // 4, V0S[c % 4]
        xq_c = np.ascontiguousarray(
            np.concatenate([x[b, v0].T, x[b, v0 + 2].T], axis=1)).astype(BFNP)
        xkv_c = np.ascontiguousarray(np.concatenate(
            [x[b, (v0 - 1) % V].T, x[b, (v0 + 1) % V].T,
             x[b, (v0 + 3) % V].T], axis=1)).astype(BFNP)
        in_maps.append({
            "xqT": xq_c, "xkvT": xkv_c,
            "wq": wq_b, "wkv": wkv_b, "wo": wo_b,
        })
    return in_maps


def kernel(x, wq, wkv, wo):
    from concourse.bass_utils import run_bass_kernel_spmd

    nc = _get_nc()
    in_maps = make_in_maps(x, wq, wkv, wo)
    res = run_bass_kernel_spmd(nc, in_maps, list(range(N_CORES)),
                               trace=False)
    out = np.empty((B, V, S, D), np.float32)
    for c in range(N_CORES):
        yc = res.results[c]["y"]
        b, v0 = c // 4, V0S[c % 4]
        out[b, v0] = yc[0:S]
        out[b, v0 + 2] = yc[S:2 * S]
    return out


# revision 25
# speedup vs baseline: 1.6021x; 1.0128x over previous
"""CrossViewAttention Trainium2 kernel (v7).

Shards the B*V=16 (batch, view) attention instances across 8 NeuronCores,
2 per core, paired as (b, v) and (b, v+2) so the two instances share KV
source view v+1 (each instance attends over views v-1, v+1 circular).
Per core the 3 distinct KV source views are projected once (25% fewer
K/V projection FLOPs). All matmul operands are bf16 (fp32 PSUM).

Schedule (single rotating 8-bank PSUM pool, no phase barriers):
  A1  K^T = wk^T @ x_kv^T   [feat, t]   (drain split ACT+DVE -> KT bf16)
  A2  V   = x_kv @ wv       [t, feat]   (+ ones col -> VA, DVE drain)
  A3+B software pipeline per head-pair j (lag 1):
      emit QK(j-1) -> emit A3(j) -> emit PV(j-1)
      so the ACT-engine exp of step j-1 hides under A3(j) matmuls.
  Softmax denominators l come free from PV's ones column. Per group the
  l row is staged at partition 0 (DVE), DMA-scattered into an
  [8-row x batch] quadrant-aligned stack, and each batch of 8 groups
  gets ONE [8,512] DVE reciprocal (HW reciprocal costs free-size*8cyc
  regardless of partition count). The 1/l rows are column-folded back
  to partition 0 by one DMA per batch; normalization (gpsimd
  partition_broadcast + in-place multiplies, all on the otherwise-idle
  Pool engine) is paced 2 groups per pipeline step so it never gates
  phase C. partition_broadcast HW quirk: dst/src partition base must
  be 0 (src column offsets are fine) - hence the fold to partition 0.
  C   y = O @ wo  (nn pairs, 8 psum banks, [128,1024] wo tiles; yt
      drains on DVE, y DMAs spread across queues)
"""
import numpy as np
import ml_dtypes

B, V, S, D = 2, 8, 256, 2048
NH, NKV, KVR = 32, 8, 2
HD = D // NH  # 64
G = NH // NKV  # 4
N_CORES = 8
P = 2  # instances per core
SCALE = 1.0 / np.sqrt(HD)
BFNP = ml_dtypes.bfloat16
V0S = (0, 1, 4, 5)  # per-core first view; pair is (v0, v0+2)

_CACHE = {}


def _build():
    import concourse.tile as tile
    import concourse.mybir as mybir
    from concourse import bacc
    from contextlib import ExitStack

    F32 = mybir.dt.float32
    BF16 = mybir.dt.bfloat16
    Exp = mybir.ActivationFunctionType.Exp
    Mult = mybir.AluOpType.mult

    nc = bacc.Bacc("TRN2", target_bir_lowering=False, debug=False,
                   num_devices=N_CORES)
    xqT = nc.dram_tensor("xqT", [D, P * S], BF16, kind="ExternalInput").ap()
    xkvT = nc.dram_tensor("xkvT", [D, 768], BF16, kind="ExternalInput").ap()
    wq = nc.dram_tensor("wq", [D, D], BF16, kind="ExternalInput").ap()
    wkv = nc.dram_tensor("wkv", [D, 1024], BF16, kind="ExternalInput").ap()
    wo = nc.dram_tensor("wo", [D, D], BF16, kind="ExternalInput").ap()
    y = nc.dram_tensor("y", [P * S, D], F32, kind="ExternalOutput").ap()

    with tile.TileContext(nc) as tc, ExitStack() as top:
        xp = top.enter_context(tc.tile_pool(name="xp", bufs=1))
        wqp = top.enter_context(tc.tile_pool(name="wqp", bufs=1))
        kvp = top.enter_context(tc.tile_pool(name="kvp", bufs=1))
        qtp = top.enter_context(tc.tile_pool(name="qtp", bufs=3))
        otp = top.enter_context(tc.tile_pool(name="otp", bufs=1))
        esp = top.enter_context(tc.tile_pool(name="esp", bufs=8))
        wsp = top.enter_context(tc.tile_pool(name="wsp", bufs=6))
        wop = top.enter_context(tc.tile_pool(name="wop", bufs=5))
        ytp = top.enter_context(tc.tile_pool(name="ytp", bufs=2))
        msc = top.enter_context(tc.tile_pool(name="msc", bufs=1))
        rsp = top.enter_context(tc.tile_pool(name="rsp", bufs=3))
        lrp = top.enter_context(tc.tile_pool(name="lrp", bufs=3))
        pp = top.enter_context(tc.tile_pool(name="pp", bufs=8, space="PSUM"))

        # ---- input DMAs ----
        # sync: xkv then (inside A2) wv.  scalar: wk.  gpsimd: xq then wq.
        # First A1 matmul needs only xkv[0] (sync) + wk[0] (scalar), which
        # issue in parallel on different queues.
        xkv = [xp.tile([128, 768], BF16, tag=f"xkv{k}", name=f"xkv{k}")
               for k in range(16)]
        wkts = []
        for k in range(16):
            nc.sync.dma_start(xkv[k][:], xkvT[k * 128:(k + 1) * 128, :])
            wkt = wsp.tile([128, 512], BF16, tag="wk", name=f"wk{k}")
            nc.scalar.dma_start(wkt[:], wkv[k * 128:(k + 1) * 128, 0:512])
            wkts.append(wkt)
        xq = [xp.tile([128, 512], BF16, tag=f"xq{k}", name=f"xq{k}")
              for k in range(16)]
        for k in range(16):
            nc.gpsimd.dma_start(xq[k][:], xqT[k * 128:(k + 1) * 128, :])
        wqsb = [wqp.tile([128, 2048], BF16, tag=f"wq{k}", name=f"wq{k}")
                for k in range(16)]
        for k in range(16):
            nc.gpsimd.dma_start(wqsb[k][:], wq[k * 128:(k + 1) * 128, :])

        # ---- persistent K^T / V_aug tiles ----
        KT = [kvp.tile([64, 768], BF16, tag=f"kt{n}", name=f"kt{n}")
              for n in range(NKV)]
        VA = [kvp.tile([128, NKV * 65], BF16, tag=f"va{t}", name=f"va{t}")
              for t in range(6)]
        for t6 in range(6):
            oc = VA[t6].rearrange("q (h c) -> q h c", c=65)[:, :, 64:65]
            nc.vector.memset(oc, 1.0)

        # ---- A1: K^T [feat, t] ----
        kps = [pp.tile([128, 512], F32, tag="pb", name=f"kps{i}")
               for i in range(8)]
        for k in range(16):
            for f in range(4):
                for h in range(2):
                    nc.tensor.matmul(kps[f * 2 + h][:, 0:384],
                                     wkts[k][:, f * 128:(f + 1) * 128],
                                     xkv[k][:, h * 384:(h + 1) * 384],
                                     start=(k == 0), stop=(k == 15))
        for f in range(4):
            for h in range(2):
                ps = kps[f * 2 + h]
                nc.scalar.copy(KT[2 * f][0:64, h * 384:(h + 1) * 384],
                               ps[0:64, 0:384])
                nc.vector.tensor_copy(
                    KT[2 * f + 1][0:64, h * 384:(h + 1) * 384],
                    ps[64:128, 0:384])

        # ---- A2: V natural [t, feat] + ones ----
        vps = [pp.tile([128, 512], F32, tag="pb", name=f"vps{t}")
               for t in range(6)]
        for k in range(16):
            wvt = wsp.tile([128, 512], BF16, tag="wv", name=f"wv{k}")
            nc.sync.dma_start(wvt[:], wkv[k * 128:(k + 1) * 128, 512:1024])
            for t6 in range(6):
                nc.tensor.matmul(vps[t6][:],
                                 xkv[k][:, t6 * 128:(t6 + 1) * 128],
                                 wvt[:],
                                 start=(k == 0), stop=(k == 15))
        for t6 in range(6):
            dst = VA[t6].rearrange("q (h c) -> q h c", c=65)[:, :, 0:64]
            src = vps[t6].rearrange("q (h c) -> q h c", c=64)
            nc.vector.tensor_copy(dst, src)

        # ---- A3 + B software pipeline ----
        OT = [[otp.tile([128, 256], BF16, tag=f"ot{p}_{j}", name=f"ot{p}_{j}")
               for j in range(16)] for p in range(P)]
        # l rows: batch b of 8 groups lives at partitions 32b..32b+7 so the
        # batched reciprocal slice is quadrant-aligned.
        lt = msc.tile([128, 512], BF16, tag="lt", name="lt")
        rt32 = msc.tile([128, 512], F32, tag="rt32", name="rt32")
        rt = msc.tile([128, 512], BF16, tag="rt", name="rt")
        # 1/l rows column-folded to partition 0: group 8b+i at cols i*512.
        rtc = msc.tile([1, 4096], BF16, tag="rtc", name="rtc")
        QTt = {}
        qk_es = {}

        def emit_a3(j):
            ps = pp.tile([128, 512], F32, tag="pb", name=f"qps{j}")
            for k in range(16):
                nc.tensor.matmul(ps[:],
                                 wqsb[k][:, j * 128:(j + 1) * 128],
                                 xq[k][:],
                                 start=(k == 0), stop=(k == 15))
            qt = qtp.tile([64, 1024], BF16, tag="qt", name=f"qt{j}")
            # qt col layout: p*512 + u*256 + s  (u = head within pair).
            # These casts run on ACT so the batched DVE reciprocal can
            # never delay them (QK of the next step needs qt).
            dst = qt[0:64].rearrange("a (p u s) -> a p u s", p=2, u=2)
            nc.scalar.copy(
                dst[:, :, 0, :], ps[0:64].rearrange("a (p s) -> a p s", p=2))
            nc.scalar.copy(
                dst[:, :, 1, :], ps[64:128].rearrange("a (p s) -> a p s", p=2))
            QTt[j] = qt

        def emit_b_qk(j):
            n = j // 2
            for p in range(P):
                g = j * 2 + p
                for tt in range(4):
                    qk = pp.tile([128, 512], F32, tag="pb",
                                 name=f"qk{g}_{tt}")
                    tcol = p * 256 + tt * 128
                    nc.tensor.matmul(qk[:],
                                     KT[n][0:64, tcol:tcol + 128],
                                     QTt[j][0:64, p * 512:(p + 1) * 512],
                                     start=True, stop=True)
                    e = esp.tile([128, 512], BF16, tag="e", name=f"e{g}_{tt}")
                    nc.scalar.activation(e[:], qk[:], Exp, scale=float(SCALE))
                    qk_es[(g, tt)] = e

        def emit_b_pv(j):
            n = j // 2
            for p in range(P):
                g = j * 2 + p
                pv = pp.tile([128, 512], F32, tag="pb", name=f"pv{g}")
                for tt in range(4):
                    nc.tensor.matmul(pv[0:65, :],
                                     VA[p * 2 + tt][:, n * 65:(n + 1) * 65],
                                     qk_es.pop((g, tt))[:],
                                     start=(tt == 0), stop=(tt == 3))
                lrow = lrp.tile([1, 512], BF16, tag="lr", name=f"lr{g}")
                with nc.allow_low_precision(reason="l sums in bf16"):
                    nc.vector.tensor_copy(lrow[:], pv[64:65, 0:512])
                qp = 32 * (g // 8) + (g % 8)
                nc.gpsimd.dma_start(lt[qp:qp + 1, :], lrow[:])
                # split the evacuation across ACT+DVE: psum drains must not
                # queue behind the (bursty) normalize multiplies on DVE.
                nc.scalar.copy(OT[p][j][0:64, :], pv[0:64, 0:256])
                nc.vector.tensor_copy(OT[p][j][64:128, :], pv[0:64, 256:512])

        def emit_recip_batch(bidx):
            base = 32 * bidx
            nc.vector.reciprocal(rt32[base:base + 8, :], lt[base:base + 8, :])
            with nc.allow_low_precision(reason="softmax denom in bf16"):
                nc.vector.tensor_copy(rt[base:base + 8, :],
                                      rt32[base:base + 8, :])
            # fold the 8 rows into columns of partition 0 (one DMA)
            nc.sync.dma_start(rtc[0:1, :], rt[base:base + 8, :])

        def emit_norm(g, tail=False):
            # NOTE: the multiplies must NOT run on gpsimd - mixing standard
            # ops with the partition_broadcast ucode on the Pool engine
            # costs a ~6us pipeline switch per transition (measured).
            j, p = g // 2, g % 2
            i = g % 8
            rsb = rsp.tile([128, 512], BF16, tag="rsb", name=f"rsb{g}")
            nc.gpsimd.partition_broadcast(
                rsb[:], rtc[0:1, i * 512:(i + 1) * 512], channels=128)
            nc.vector.tensor_tensor(OT[p][j][0:64, :], OT[p][j][0:64, :],
                                    rsb[0:64, 0:256], Mult)
            nc.vector.tensor_tensor(OT[p][j][64:128, :], OT[p][j][64:128, :],
                                    rsb[64:128, 256:512], Mult)

        pending = []
        for j in range(17):
            if j >= 1:
                emit_b_qk(j - 1)
            if j < 16:
                emit_a3(j)
            if j >= 1:
                emit_b_pv(j - 1)
            # drain norms BEFORE a new recip/fold: the fold overwrites rtc,
            # so all reads of the previous batch must already be emitted
            # (the WAR dep then orders the fold after them).
            for _ in range(2):
                if pending:
                    emit_norm(pending.pop(0))
            if j >= 1 and (j - 1) % 4 == 3:
                bidx = (j - 1) // 4
                emit_recip_batch(bidx)
                pending.extend(range(8 * bidx, 8 * bidx + 8))
        # tail: broadcasts stay on Pool, multiplies go to the now-idle DVE
        while pending:
            emit_norm(pending.pop(0), tail=True)

        # ---- C: y = O @ wo  (nn pairs, 8 psum banks, [128,1024] wo) ----
        for half in range(2):
            acc = [pp.tile([128, 512], F32, tag="pb", name=f"acc{half}_{i}")
                   for i in range(8)]
            for k in range(16):
                wot = wop.tile([128, 1024], BF16, tag="wo",
                               name=f"wo{half}_{k}")
                nc.sync.dma_start(
                    wot[:],
                    wo[k * 128:(k + 1) * 128, half * 1024:(half + 1) * 1024])
                for n2 in range(2):
                    for p in range(P):
                        for m in range(2):
                            nc.tensor.matmul(
                                acc[n2 * 4 + p * 2 + m][:],
                                OT[p][k][:, m * 128:(m + 1) * 128],
                                wot[:, n2 * 512:(n2 + 1) * 512],
                                start=(k == 0), stop=(k == 15))
            for i, (p, m) in enumerate([(0, 0), (0, 1), (1, 0), (1, 1)]):
                yt = ytp.tile([128, 1024], F32, tag="yt",
                              name=f"yt{half}_{p}_{m}")
                for n2 in range(2):
                    eng = nc.vector if (i + n2) % 2 == 0 else nc.scalar
                    if eng is nc.vector:
                        eng.tensor_copy(yt[:, n2 * 512:(n2 + 1) * 512],
                                        acc[n2 * 4 + p * 2 + m][:])
                    else:
                        eng.copy(yt[:, n2 * 512:(n2 + 1) * 512],
                                 acc[n2 * 4 + p * 2 + m][:])
                r0 = p * 256 + m * 128
                q = (nc.sync, nc.gpsimd, nc.scalar, nc.sync)[i]
                q.dma_start(
                    y[r0:r0 + 128, half * 1024:(half + 1) * 1024], yt[:])

    nc.compile()
    return nc


def _get_nc():
    if "nc" not in _CACHE:
        _CACHE["nc"] = _build()
    return _CACHE["nc"]


def make_in_maps(x, wq, wkv, wo):
    x = np.asarray(x, dtype=np.float32)
    wq_b = np.asarray(wq, dtype=BFNP)
    wkv_b = np.asarray(wkv, dtype=BFNP)
    wo_b = np.asarray(wo, dtype=BFNP)
    in_maps = []
    for c in range(N_CORES):
        b, v0 = c // 4, V0S[c % 4]
        xq_c = np.ascontiguousarray(
            np.concatenate([x[b, v0].T, x[b, v0 + 2].T], axis=1)).astype(BFNP)
        xkv_c = np.ascontiguousarray(np.concatenate(
            [x[b, (v0 - 1) % V].T, x[b, (v0 + 1) % V].T,
             x[b, (v0 + 3) % V].T], axis=1)).astype(BFNP)
        in_maps.append({
            "xqT": xq_c, "xkvT": xkv_c,
            "wq": wq_b, "wkv": wkv_b, "wo": wo_b,
        })
    return in_maps


def kernel(x, wq, wkv, wo):
    from concourse.bass_utils import run_bass_kernel_spmd

    nc = _get_nc()
    in_maps = make_in_maps(x, wq, wkv, wo)
    res = run_bass_kernel_spmd(nc, in_maps, list(range(N_CORES)),
                               trace=False)
    out = np.empty((B, V, S, D), np.float32)
    for c in range(N_CORES):
        yc = res.results[c]["y"]
        b, v0 = c // 4, V0S[c % 4]
        out[b, v0] = yc[0:S]
        out[b, v0 + 2] = yc[S:2 * S]
    return out


# revision 27
# speedup vs baseline: 1.6345x; 1.0202x over previous
"""CrossViewAttention Trainium2 kernel (v7).

Shards the B*V=16 (batch, view) attention instances across 8 NeuronCores,
2 per core, paired as (b, v) and (b, v+2) so the two instances share KV
source view v+1 (each instance attends over views v-1, v+1 circular).
Per core the 3 distinct KV source views are projected once (25% fewer
K/V projection FLOPs). All matmul operands are bf16 (fp32 PSUM).

Schedule (single rotating 8-bank PSUM pool, no phase barriers):
  A1  K^T = wk^T @ x_kv^T   [feat, t]   (drain split ACT+DVE -> KT bf16)
  A2  V   = x_kv @ wv       [t, feat]   (+ ones col -> VA, DVE drain)
  A3+B software pipeline per head-pair j (lag 1):
      emit QK(j-1) -> emit A3(j) -> emit PV(j-1)
      so the ACT-engine exp of step j-1 hides under A3(j) matmuls.
  Softmax denominators l come free from PV's ones column. Per group the
  l row is staged at partition 0 (DVE), DMA-scattered into an
  [8-row x batch] quadrant-aligned stack, and each batch of 8 groups
  gets ONE [8,512] DVE reciprocal (HW reciprocal costs free-size*8cyc
  regardless of partition count). The 1/l rows are column-folded back
  to partition 0 by one DMA per batch; normalization (gpsimd
  partition_broadcast + in-place multiplies, all on the otherwise-idle
  Pool engine) is paced 2 groups per pipeline step so it never gates
  phase C. partition_broadcast HW quirk: dst/src partition base must
  be 0 (src column offsets are fine) - hence the fold to partition 0.
  C   y = O @ wo  (nn pairs, 8 psum banks, [128,1024] wo tiles; yt
      drains on DVE, y DMAs spread across queues)
"""
import numpy as np
import ml_dtypes

B, V, S, D = 2, 8, 256, 2048
NH, NKV, KVR = 32, 8, 2
HD = D // NH  # 64
G = NH // NKV  # 4
N_CORES = 8
P = 2  # instances per core
SCALE = 1.0 / np.sqrt(HD)
BFNP = ml_dtypes.bfloat16
V0S = (0, 1, 4, 5)  # per-core first view; pair is (v0, v0+2)

_CACHE = {}


def _build():
    import concourse.tile as tile
    import concourse.mybir as mybir
    from concourse import bacc
    from contextlib import ExitStack

    F32 = mybir.dt.float32
    BF16 = mybir.dt.bfloat16
    Exp = mybir.ActivationFunctionType.Exp
    Mult = mybir.AluOpType.mult

    nc = bacc.Bacc("TRN2", target_bir_lowering=False, debug=False,
                   num_devices=N_CORES)
    xqT = nc.dram_tensor("xqT", [D, P * S], BF16, kind="ExternalInput").ap()
    xkvT = nc.dram_tensor("xkvT", [D, 768], BF16, kind="ExternalInput").ap()
    wq = nc.dram_tensor("wq", [D, D], BF16, kind="ExternalInput").ap()
    wkv = nc.dram_tensor("wkv", [D, 1024], BF16, kind="ExternalInput").ap()
    wo = nc.dram_tensor("wo", [D, D], BF16, kind="ExternalInput").ap()
    y = nc.dram_tensor("y", [P * S, D], F32, kind="ExternalOutput").ap()

    with tile.TileContext(nc) as tc, ExitStack() as top:
        xp = top.enter_context(tc.tile_pool(name="xp", bufs=1))
        wqp = top.enter_context(tc.tile_pool(name="wqp", bufs=1))
        kvp = top.enter_context(tc.tile_pool(name="kvp", bufs=1))
        qtp = top.enter_context(tc.tile_pool(name="qtp", bufs=3))
        otp = top.enter_context(tc.tile_pool(name="otp", bufs=1))
        esp = top.enter_context(tc.tile_pool(name="esp", bufs=8))
        wsp = top.enter_context(tc.tile_pool(name="wsp", bufs=6))
        wop = top.enter_context(tc.tile_pool(name="wop", bufs=4))
        ytp = top.enter_context(tc.tile_pool(name="ytp", bufs=2))
        msc = top.enter_context(tc.tile_pool(name="msc", bufs=1))
        rsp = top.enter_context(tc.tile_pool(name="rsp", bufs=3))
        lrp = top.enter_context(tc.tile_pool(name="lrp", bufs=3))
        pp = top.enter_context(tc.tile_pool(name="pp", bufs=8, space="PSUM"))

        # ---- input DMAs ----
        # sync: xkv then (inside A2) wv.  scalar: wk.  gpsimd: xq then wq.
        # First A1 matmul needs only xkv[0] (sync) + wk[0] (scalar), which
        # issue in parallel on different queues.
        xkv = [xp.tile([128, 768], BF16, tag=f"xkv{k}", name=f"xkv{k}")
               for k in range(16)]
        wkts = []
        for k in range(16):
            nc.sync.dma_start(xkv[k][:], xkvT[k * 128:(k + 1) * 128, :])
            wkt = wsp.tile([128, 512], BF16, tag="wk", name=f"wk{k}")
            nc.scalar.dma_start(wkt[:], wkv[k * 128:(k + 1) * 128, 0:512])
            wkts.append(wkt)
        xq = [xp.tile([128, 512], BF16, tag=f"xq{k}", name=f"xq{k}")
              for k in range(16)]
        for k in range(16):
            nc.gpsimd.dma_start(xq[k][:], xqT[k * 128:(k + 1) * 128, :])
        wqsb = [wqp.tile([128, 2048], BF16, tag=f"wq{k}", name=f"wq{k}")
                for k in range(16)]
        for k in range(16):
            nc.gpsimd.dma_start(wqsb[k][:], wq[k * 128:(k + 1) * 128, :])

        # ---- persistent K^T / V_aug tiles ----
        KT = [kvp.tile([64, 768], BF16, tag=f"kt{n}", name=f"kt{n}")
              for n in range(NKV)]
        VA = [kvp.tile([128, NKV * 65], BF16, tag=f"va{t}", name=f"va{t}")
              for t in range(6)]
        for t6 in range(6):
            oc = VA[t6].rearrange("q (h c) -> q h c", c=65)[:, :, 64:65]
            nc.vector.memset(oc, 1.0)

        # ---- A1: K^T [feat, t] ----
        kps = [pp.tile([128, 512], F32, tag="pb", name=f"kps{i}")
               for i in range(8)]
        for k in range(16):
            for f in range(4):
                for h in range(2):
                    nc.tensor.matmul(kps[f * 2 + h][:, 0:384],
                                     wkts[k][:, f * 128:(f + 1) * 128],
                                     xkv[k][:, h * 384:(h + 1) * 384],
                                     start=(k == 0), stop=(k == 15))
        for f in range(4):
            for h in range(2):
                ps = kps[f * 2 + h]
                nc.scalar.copy(KT[2 * f][0:64, h * 384:(h + 1) * 384],
                               ps[0:64, 0:384])
                nc.vector.tensor_copy(
                    KT[2 * f + 1][0:64, h * 384:(h + 1) * 384],
                    ps[64:128, 0:384])

        # ---- A2: V natural [t, feat] + ones ----
        vps = [pp.tile([128, 512], F32, tag="pb", name=f"vps{t}")
               for t in range(6)]
        for k in range(16):
            wvt = wsp.tile([128, 512], BF16, tag="wv", name=f"wv{k}")
            nc.sync.dma_start(wvt[:], wkv[k * 128:(k + 1) * 128, 512:1024])
            for t6 in range(6):
                nc.tensor.matmul(vps[t6][:],
                                 xkv[k][:, t6 * 128:(t6 + 1) * 128],
                                 wvt[:],
                                 start=(k == 0), stop=(k == 15))
        for t6 in range(6):
            dst = VA[t6].rearrange("q (h c) -> q h c", c=65)[:, :, 0:64]
            src = vps[t6].rearrange("q (h c) -> q h c", c=64)
            nc.vector.tensor_copy(dst, src)

        # ---- A3 + B software pipeline ----
        OT = [[otp.tile([128, 256], BF16, tag=f"ot{p}_{j}", name=f"ot{p}_{j}")
               for j in range(16)] for p in range(P)]
        # l rows: batch b of 8 groups lives at partitions 32b..32b+7 so the
        # batched reciprocal slice is quadrant-aligned.
        lt = msc.tile([128, 512], BF16, tag="lt", name="lt")
        rt32 = msc.tile([128, 512], F32, tag="rt32", name="rt32")
        rt = msc.tile([128, 512], BF16, tag="rt", name="rt")
        # 1/l rows column-folded to partition 0: group 8b+i at cols i*512.
        # Double-buffered per batch so a new fold never has to wait for the
        # previous batch's (late-running) broadcast reads.
        rtcs = [msc.tile([1, 4096], BF16, tag=f"rtc{i}", name=f"rtc{i}")
                for i in range(2)]
        QTt = {}
        qk_es = {}

        def emit_a3(j):
            ps = pp.tile([128, 512], F32, tag="pb", name=f"qps{j}")
            for k in range(16):
                nc.tensor.matmul(ps[:],
                                 wqsb[k][:, j * 128:(j + 1) * 128],
                                 xq[k][:],
                                 start=(k == 0), stop=(k == 15))
            qt = qtp.tile([64, 1024], BF16, tag="qt", name=f"qt{j}")
            # qt col layout: p*512 + u*256 + s  (u = head within pair).
            # These casts run on ACT so the batched DVE reciprocal can
            # never delay them (QK of the next step needs qt).
            dst = qt[0:64].rearrange("a (p u s) -> a p u s", p=2, u=2)
            nc.scalar.copy(
                dst[:, :, 0, :], ps[0:64].rearrange("a (p s) -> a p s", p=2))
            nc.scalar.copy(
                dst[:, :, 1, :], ps[64:128].rearrange("a (p s) -> a p s", p=2))
            QTt[j] = qt

        def emit_b_qk(j):
            n = j // 2
            for p in range(P):
                g = j * 2 + p
                for tt in range(4):
                    qk = pp.tile([128, 512], F32, tag="pb",
                                 name=f"qk{g}_{tt}")
                    tcol = p * 256 + tt * 128
                    nc.tensor.matmul(qk[:],
                                     KT[n][0:64, tcol:tcol + 128],
                                     QTt[j][0:64, p * 512:(p + 1) * 512],
                                     start=True, stop=True)
                    e = esp.tile([128, 512], BF16, tag="e", name=f"e{g}_{tt}")
                    nc.scalar.activation(e[:], qk[:], Exp, scale=float(SCALE))
                    qk_es[(g, tt)] = e

        def emit_b_pv(j):
            n = j // 2
            for p in range(P):
                g = j * 2 + p
                pv = pp.tile([128, 512], F32, tag="pb", name=f"pv{g}")
                for tt in range(4):
                    nc.tensor.matmul(pv[0:65, :],
                                     VA[p * 2 + tt][:, n * 65:(n + 1) * 65],
                                     qk_es.pop((g, tt))[:],
                                     start=(tt == 0), stop=(tt == 3))
                lrow = lrp.tile([1, 512], BF16, tag="lr", name=f"lr{g}")
                with nc.allow_low_precision(reason="l sums in bf16"):
                    nc.vector.tensor_copy(lrow[:], pv[64:65, 0:512])
                qp = 32 * (g // 8) + (g % 8)
                nc.gpsimd.dma_start(lt[qp:qp + 1, :], lrow[:])
                # split the evacuation across ACT+DVE: psum drains must not
                # queue behind the (bursty) normalize multiplies on DVE.
                nc.scalar.copy(OT[p][j][0:64, :], pv[0:64, 0:256])
                nc.vector.tensor_copy(OT[p][j][64:128, :], pv[0:64, 256:512])

        def emit_recip_batch(bidx):
            base = 32 * bidx
            nc.vector.reciprocal(rt32[base:base + 8, :], lt[base:base + 8, :])
            with nc.allow_low_precision(reason="softmax denom in bf16"):
                nc.vector.tensor_copy(rt[base:base + 8, :],
                                      rt32[base:base + 8, :])
            # fold the 8 rows into columns of partition 0 (one DMA)
            nc.sync.dma_start(rtcs[bidx % 2][0:1, :], rt[base:base + 8, :])

        def emit_norm(g, tail=False):
            # NOTE: the multiplies must NOT run on gpsimd - mixing standard
            # ops with the partition_broadcast ucode on the Pool engine
            # costs a ~6us pipeline switch per transition (measured).
            j, p = g // 2, g % 2
            i = g % 8
            rtc = rtcs[(g // 8) % 2]
            rsbA = rsp.tile([128, 256], BF16, tag="rsbA", name=f"rsbA{g}")
            nc.gpsimd.partition_broadcast(
                rsbA[:], rtc[0:1, i * 512:i * 512 + 256], channels=128)
            rsbB = rsp.tile([128, 256], BF16, tag="rsbB", name=f"rsbB{g}")
            nc.gpsimd.partition_broadcast(
                rsbB[:], rtc[0:1, i * 512 + 256:(i + 1) * 512], channels=128)
            nc.vector.tensor_tensor(OT[p][j][0:64, :], OT[p][j][0:64, :],
                                    rsbA[0:64, :], Mult)
            nc.vector.tensor_tensor(OT[p][j][64:128, :], OT[p][j][64:128, :],
                                    rsbB[64:128, :], Mult)

        # The recip->convert->fold->broadcast chain spans 3 engines + 2
        # DMAs (~10us latency): norms drain only 2+ iterations after their
        # batch's fold was emitted, so the TT never head-blocks the DVE
        # queue. Whatever is left drains lazily inside C's k-loop.
        pending = []   # (group, ready_iter)
        for j in range(17):
            if j >= 1:
                emit_b_qk(j - 1)
            if j < 16:
                emit_a3(j)
            if j >= 1:
                emit_b_pv(j - 1)
            drained = 0
            while pending and pending[0][1] <= j and drained < 3:
                emit_norm(pending.pop(0)[0])
                drained += 1
            if j >= 1 and (j - 1) % 4 == 3:
                bidx = (j - 1) // 4
                emit_recip_batch(bidx)
                pending.extend((g, j + 2)
                               for g in range(8 * bidx, 8 * bidx + 8))

        # ---- C: y = O @ wo  (nn pairs, 8 psum banks, [128,1024] wo) ----
        for half in range(2):
            acc = [pp.tile([128, 512], F32, tag="pb", name=f"acc{half}_{i}")
                   for i in range(8)]
            for k in range(16):
                if half == 0:
                    while pending and pending[0][0] <= 2 * k + 1:
                        emit_norm(pending.pop(0)[0], tail=True)
                wot = wop.tile([128, 1024], BF16, tag="wo",
                               name=f"wo{half}_{k}")
                nc.sync.dma_start(
                    wot[:],
                    wo[k * 128:(k + 1) * 128, half * 1024:(half + 1) * 1024])
                for n2 in range(2):
                    for p in range(P):
                        for m in range(2):
                            nc.tensor.matmul(
                                acc[n2 * 4 + p * 2 + m][:],
                                OT[p][k][:, m * 128:(m + 1) * 128],
                                wot[:, n2 * 512:(n2 + 1) * 512],
                                start=(k == 0), stop=(k == 15))
            for i, (p, m) in enumerate([(0, 0), (0, 1), (1, 0), (1, 1)]):
                yt = ytp.tile([128, 1024], F32, tag="yt",
                              name=f"yt{half}_{p}_{m}")
                for n2 in range(2):
                    eng = nc.vector if (i + n2) % 2 == 0 else nc.scalar
                    if eng is nc.vector:
                        eng.tensor_copy(yt[:, n2 * 512:(n2 + 1) * 512],
                                        acc[n2 * 4 + p * 2 + m][:])
                    else:
                        eng.copy(yt[:, n2 * 512:(n2 + 1) * 512],
                                 acc[n2 * 4 + p * 2 + m][:])
                r0 = p * 256 + m * 128
                q = (nc.sync, nc.gpsimd, nc.scalar, nc.sync)[i]
                q.dma_start(
                    y[r0:r0 + 128, half * 1024:(half + 1) * 1024], yt[:])

    nc.compile()
    return nc


def _get_nc():
    if "nc" not in _CACHE:
        _CACHE["nc"] = _build()
    return _CACHE["nc"]


def make_in_maps(x, wq, wkv, wo):
    x = np.asarray(x, dtype=np.float32)
    wq_b = np.asarray(wq, dtype=BFNP)
    wkv_b = np.asarray(wkv, dtype=BFNP)
    wo_b = np.asarray(wo, dtype=BFNP)
    in_maps = []
    for c in range(N_CORES):
        b, v0 = c // 4, V0S[c % 4]
        xq_c = np.ascontiguousarray(
            np.concatenate([x[b, v0].T, x[b, v0 + 2].T], axis=1)).astype(BFNP)
        xkv_c = np.ascontiguousarray(np.concatenate(
            [x[b, (v0 - 1) % V].T, x[b, (v0 + 1) % V].T,
             x[b, (v0 + 3) % V].T], axis=1)).astype(BFNP)
        in_maps.append({
            "xqT": xq_c, "xkvT": xkv_c,
            "wq": wq_b, "wkv": wkv_b, "wo": wo_b,
        })
    return in_maps


def kernel(x, wq, wkv, wo):
    from concourse.bass_utils import run_bass_kernel_spmd

    nc = _get_nc()
    in_maps = make_in_maps(x, wq, wkv, wo)
    res = run_bass_kernel_spmd(nc, in_maps, list(range(N_CORES)),
                               trace=False)
    out = np.empty((B, V, S, D), np.float32)
    for c in range(N_CORES):
        yc = res.results[c]["y"]
        b, v0 = c // 4, V0S[c % 4]
        out[b, v0] = yc[0:S]
        out[b, v0 + 2] = yc[S:2 * S]
    return out
